# revision 8
# baseline (speedup 1.0000x reference)
# Additive attention kernel for 8 Trainium2 NeuronCores.
#
# reference:
#   q_lin = q @ Wq_w.T + Wq_b                    [B,1,H]
#   k_lin = k @ Wk_w.T + Wk_b                    [B,S,H]
#   scores = tanh(q_lin + k_lin) @ v_w[0]        [B,S]
#   attn = softmax(where(mask, scores, -1e9))    [B,S]
#   ctx = attn @ v                               [B,H]
#   returns (ctx, attn)
#
# Sharding: data-parallel over B. 16 batches / 8 cores = 2 per core. No
# collectives. Host pre-transposes k and v to [B,H,S] so the contraction dim
# (h) lands on SBUF partitions with fully contiguous DMA, and folds the whole
# q-path (q @ Wq_w.T + Wq_b + Wk_b) into a per-(b,g) bias vector that the
# ScalarE applies inside the tanh activation.
#
# Device pipeline per batch (zT layout [g,s]):
#   zT[g,s] = sum_h WkT[h,g] * kT[h,s]     PE, float32r, N=512 chunks
#   th[g,s] = tanh(zT + qkb[g])            ScalarE, per-partition bias
#   scores[s] = sum_g vw[g] * th[g,s]      PE, M=1 matmuls accumulated
#   attn = exp(scores+madd)/sum            ScalarE exp w/ accum + DVE
#   bc[p,s] = attn[s] (all partitions)     PE ones-matmul broadcast
#   ctx[h] = sum_s vT[h,s]*bc[.,s]         DVE tensor_tensor_reduce
#
# Softmax skips max-subtraction: |scores| <= sum|v_w| <= H*(1/32) = 32, so
# exp() cannot overflow fp32 and matches the reference to fp32 rounding.

import os
import sys

import numpy as np

for _p in ("/opt/trn_rl_repo", os.path.expanduser("~/.axon_site/_ro/trn_rl_repo")):
    if os.path.isdir(_p) and _p not in sys.path:
        sys.path.insert(0, _p)

import concourse.bass as bass  # noqa: E402
import concourse.mybir as mybir  # noqa: E402
import concourse.tile as tile  # noqa: E402
from concourse import bacc  # noqa: E402
from concourse.bass_utils import run_bass_kernel_spmd  # noqa: E402
from concourse.masks import make_identity  # noqa: E402

F32 = mybir.dt.float32
F32R = mybir.dt.float32r
BF16 = mybir.dt.bfloat16
AF = mybir.ActivationFunctionType
ALU = mybir.AluOpType

B, S, H = 16, 4096, 1024
NCORES = 8
BPC = B // NCORES  # batches per core
SC = 512  # s-chunk (one PSUM bank at fp32)
NSC = S // SC  # 8
NHC = H // 128  # 8 contraction chunks
NGC = H // 128  # 8 output (g) chunks

_CACHE = {}
LAST_RESULTS = None


def _build():
    nc = bacc.Bacc(
        "TRN2", target_bir_lowering=False, debug=False, num_devices=NCORES
    )
    kT = nc.dram_tensor("kt_in", [BPC, H, S], F32, kind="ExternalInput").ap()
    vT = nc.dram_tensor("vt_in", [BPC, H, S], F32, kind="ExternalInput").ap()
    wkT = nc.dram_tensor("wkt_in", [H, H], F32, kind="ExternalInput").ap()
    vw = nc.dram_tensor("vw_in", [H], F32, kind="ExternalInput").ap()
    qkb = nc.dram_tensor("qkb_in", [BPC, H], F32, kind="ExternalInput").ap()
    madd = nc.dram_tensor("madd_in", [BPC, S], F32, kind="ExternalInput").ap()
    ctx_o = nc.dram_tensor("ctx_out", [BPC, H], F32, kind="ExternalOutput").ap()
    attn_o = nc.dram_tensor("attn_out", [BPC, S], F32, kind="ExternalOutput").ap()

    with tile.TileContext(nc) as tc:
        with (
            tc.tile_pool(name="consts", bufs=1) as consts,
            tc.tile_pool(name="kpool", bufs=2) as kpool,
            tc.tile_pool(name="vpool", bufs=6) as vpool,
            tc.tile_pool(name="thpool", bufs=3) as thpool,
            tc.tile_pool(name="rows", bufs=2) as rows,
            tc.tile_pool(name="bcpool", bufs=8) as bcpool,
            tc.tile_pool(name="misc", bufs=2) as misc,
            tc.tile_pool(name="pz", bufs=2, space="PSUM") as pz,
            tc.tile_pool(name="ps", bufs=2, space="PSUM") as ps,
            tc.tile_pool(name="pb", bufs=2, space="PSUM") as pb,
            tc.tile_pool(name="pt", bufs=1, space="PSUM") as pt,
        ):
            # ---- constants ----
            wk_sb = consts.tile([128, NHC, H], F32R, name="wk_sb")
            nc.sync.dma_start(
                out=wk_sb,
                in_=wkT.rearrange("(hc p) g -> p hc g", p=128).bitcast(F32R),
            )
            vw_sb = consts.tile([128, NGC], F32, name="vw_sb")
            nc.sync.dma_start(out=vw_sb, in_=vw.rearrange("(gc p) -> p gc", p=128))
            vw_bf = consts.tile([128, NGC], BF16, name="vw_bf")
            nc.vector.tensor_copy(vw_bf, vw_sb)
            qkb_sb = consts.tile([128, BPC, NGC], F32, name="qkb_sb")
            nc.sync.dma_start(
                out=qkb_sb, in_=qkb.rearrange("b (gc p) -> p b gc", p=128)
            )
            ident = consts.tile([128, 128], F32, name="ident")
            make_identity(nc, ident)
            ones_sb = consts.tile([1, 128], F32, name="ones_sb")
            nc.vector.memset(ones_sb, 1.0)
            ctx_cols = consts.tile([128, BPC, NHC], F32, name="ctx_cols")

            # Deferred PE work: score / broadcast matmuls depend on ScalarE /
            # DVE results; emitting them a little late keeps the in-order PE
            # queue from stalling on cross-engine latency.
            pending = []

            def pop_pending():
                if pending:
                    pending.pop(0)()

            for b in range(BPC):
                scores = rows.tile([1, S], F32, tag="scores", name=f"scores_{b}")
                madd_sb = rows.tile([1, S], F32, tag="madd", name=f"madd_{b}")
                nc.sync.dma_start(out=madd_sb, in_=madd[b][None])
                parts_b = [
                    misc.tile(
                        [128, NSC], F32, tag="parts", bufs=2 * NHC,
                        name=f"parts_{b}_{hc}",
                    )
                    for hc in range(NHC)
                ]

                for sc in range(NSC):
                    kt = kpool.tile([128, NHC, SC], F32R, tag="kt", name=f"kt_{b}_{sc}")
                    nc.sync.dma_start(
                        out=kt,
                        in_=kT[b, :, sc * SC:(sc + 1) * SC].rearrange(
                            "(hc p) s -> p hc s", p=128
                        ).bitcast(F32R),
                    )
                    sc_ps = ps.tile([1, SC], F32, tag="scps", name=f"scps_{b}_{sc}")
                    for gc in range(NGC):
                        z_ps = pz.tile(
                            [128, SC], F32, tag="zps", name=f"z_{b}_{sc}_{gc}"
                        )
                        for hc in range(NHC):
                            nc.tensor.matmul(
                                z_ps,
                                lhsT=wk_sb[:, hc, gc * 128:(gc + 1) * 128],
                                rhs=kt[:, hc, :],
                                start=(hc == 0),
                                stop=(hc == NHC - 1),
                            )
                        th = thpool.tile(
                            [128, SC], BF16, tag="th", name=f"th_{b}_{sc}_{gc}"
                        )
                        nc.scalar.activation(
                            th, z_ps, AF.Tanh, bias=qkb_sb[:, b, gc:gc + 1], scale=1.0
                        )

                        def score_mm(b=b, sc=sc, gc=gc, th=th, sc_ps=sc_ps,
                                     scores=scores, madd_sb=madd_sb):
                            nc.tensor.matmul(
                                sc_ps,
                                lhsT=vw_bf[:, gc:gc + 1],
                                rhs=th,
                                start=(gc == 0),
                                stop=(gc == NGC - 1),
                                skip_group_check=True,
                            )
                            if gc == NGC - 1:
                                nc.vector.tensor_add(
                                    scores[:, sc * SC:(sc + 1) * SC],
                                    sc_ps,
                                    madd_sb[:, sc * SC:(sc + 1) * SC],
                                )

                        pop_pending()
                        pending.append(score_mm)

                def softmax_item(b=b, scores=scores):
                    denom = rows.tile([1, 1], F32, tag="denom", name=f"den_{b}")
                    nc.scalar.activation(scores, scores, AF.Exp, accum_out=denom)
                    recip = rows.tile([1, 1], F32, tag="recip", name=f"rec_{b}")
                    nc.vector.reciprocal(recip, denom)
                    nc.vector.tensor_scalar_mul(scores, scores, recip)
                    nc.sync.dma_start(out=attn_o[b][None], in_=scores)

                pending.append(softmax_item)

                for sc in range(NSC):
                    def ctx_item(b=b, sc=sc, scores=scores, parts_b=parts_b):
                        bc_ps = pb.tile(
                            [128, SC], F32, tag="bcps", name=f"bcps_{b}_{sc}"
                        )
                        nc.tensor.matmul(
                            bc_ps,
                            lhsT=ones_sb,
                            rhs=scores[:, sc * SC:(sc + 1) * SC],
                            start=True,
                            stop=True,
                        )
                        bc_sb = bcpool.tile(
                            [128, SC], F32, tag="bc", name=f"bc_{b}_{sc}"
                        )
                        nc.scalar.copy(bc_sb, bc_ps)
                        for hc in range(NHC):
                            vt = vpool.tile(
                                [128, SC], F32, tag="vt", name=f"vt_{b}_{sc}_{hc}"
                            )
                            nc.sync.dma_start(
                                out=vt,
                                in_=vT[b, hc * 128:(hc + 1) * 128,
                                       sc * SC:(sc + 1) * SC],
                            )
                            prod = vpool.tile(
                                [128, SC], F32, tag="prod", bufs=3,
                                name=f"prod_{b}_{sc}_{hc}",
                            )
                            nc.vector.tensor_mul(prod, vt, bc_sb)
                            junk = vpool.tile(
                                [128, SC], F32, tag="junk", bufs=2,
                                name=f"junk_{b}_{sc}_{hc}",
                            )
                            # ScalarE pass exists only for its running-sum
                            # side output (the s-reduction of attn*v).
                            nc.scalar.activation(
                                junk, prod, AF.Copy,
                                accum_out=parts_b[hc][:, sc:sc + 1],
                            )
                        if sc == NSC - 1:
                            for hc in range(NHC):
                                nc.vector.reduce_sum(
                                    ctx_cols[:, b, hc:hc + 1],
                                    parts_b[hc],
                                    axis=mybir.AxisListType.X,
                                )

                    pending.append(ctx_item)

            while pending:
                pending.pop(0)()

            # ctx_cols [128p, (b,hc)] -> [16, 128] so the DRAM store is
            # 16 contiguous 512B rows instead of a 4B-scatter.
            ct_ps = pt.tile([16, 128], F32, name="ct_ps")
            nc.tensor.transpose(
                ct_ps, ctx_cols.rearrange("p b h -> p (b h)"), ident
            )
            ctx_row = consts.tile([16, 128], F32, name="ctx_row")
            nc.vector.tensor_copy(ctx_row, ct_ps)
            nc.sync.dma_start(
                out=ctx_o.rearrange("b (hc j) -> (b hc) j", j=128), in_=ctx_row
            )

    nc.compile()
    return nc


def get_nc():
    if "nc" not in _CACHE:
        _CACHE["nc"] = _build()
    return _CACHE["nc"]


def make_in_maps(q, k, v, mask, Wq_w, Wq_b, Wk_w, Wk_b, v_w):
    q = np.asarray(q, np.float32)
    k = np.asarray(k, np.float32)
    v = np.asarray(v, np.float32)
    mask = np.asarray(mask)
    Wq_w = np.asarray(Wq_w, np.float32)
    Wq_b = np.asarray(Wq_b, np.float32)
    Wk_w = np.asarray(Wk_w, np.float32)
    Wk_b = np.asarray(Wk_b, np.float32)
    v_w = np.asarray(v_w, np.float32)

    # whole q path on host: [B,H], folded with both biases
    qkb = q[:, 0, :] @ Wq_w.T + Wq_b + Wk_b
    kT = np.ascontiguousarray(k.transpose(0, 2, 1))
    vT = np.ascontiguousarray(v.transpose(0, 2, 1))
    wkT = np.ascontiguousarray(Wk_w.T)
    madd = np.where(mask, np.float32(0.0), np.float32(-1e9)).astype(np.float32)
    vww = np.ascontiguousarray(v_w[0])

    in_maps = []
    for c in range(NCORES):
        sl = slice(c * BPC, (c + 1) * BPC)
        in_maps.append(
            {
                "kt_in": np.ascontiguousarray(kT[sl]),
                "vt_in": np.ascontiguousarray(vT[sl]),
                "wkt_in": wkT,
                "vw_in": vww,
                "qkb_in": np.ascontiguousarray(qkb[sl]),
                "madd_in": np.ascontiguousarray(madd[sl]),
            }
        )
    return in_maps


def kernel(q, k, v, mask, Wq_w, Wq_b, Wk_w, Wk_b, v_w):
    global LAST_RESULTS
    in_maps = make_in_maps(q, k, v, mask, Wq_w, Wq_b, Wk_w, Wk_b, v_w)
    nc = get_nc()
    res = run_bass_kernel_spmd(nc, in_maps, core_ids=list(range(NCORES)))
    LAST_RESULTS = res
    ctx = np.concatenate(
        [res.results[c]["ctx_out"] for c in range(NCORES)], axis=0
    ).astype(np.float32)
    attn = np.concatenate(
        [res.results[c]["attn_out"] for c in range(NCORES)], axis=0
    ).astype(np.float32)
    return ctx, attn


# revision 17
# speedup vs baseline: 1.2126x; 1.2126x over previous
# Additive attention kernel for 8 Trainium2 NeuronCores.
#
# reference:
#   q_lin = q @ Wq_w.T + Wq_b                    [B,1,H]
#   k_lin = k @ Wk_w.T + Wk_b                    [B,S,H]
#   scores = tanh(q_lin + k_lin) @ v_w[0]        [B,S]
#   attn = softmax(where(mask, scores, -1e9))    [B,S]
#   ctx = attn @ v                               [B,H]
#   returns (ctx, attn)
#
# Sharding: data-parallel over B. 16 batches / 8 cores = 2 per core. No
# collectives. Host pre-transposes k and v to [B,H,S] so the contraction dim
# (h) lands on SBUF partitions with fully contiguous DMA, and folds the whole
# q-path (q @ Wq_w.T + Wq_b + Wk_b) into a per-(b,g) bias vector that the
# ScalarE applies inside the tanh activation.
#
# Device pipeline per batch (zT layout [g,s]):
#   zT[g,s] = sum_h WkT[h,g] * kT[h,s]     PE, float32r, N=512 chunks
#   th[g,s] = tanh(zT + qkb[g])            ScalarE, per-partition bias
#   scores[s] = sum_g vw[g] * th[g,s]      PE, M=1 matmuls accumulated
#   attn = exp(scores+madd)/sum            ScalarE exp w/ accum + DVE
#   bc[p,s] = attn[s] (all partitions)     PE ones-matmul broadcast
#   ctx[h] = sum_s vT[h,s]*bc[.,s]         DVE tensor_tensor_reduce
#
# Softmax skips max-subtraction: |scores| <= sum|v_w| <= H*(1/32) = 32, so
# exp() cannot overflow fp32 and matches the reference to fp32 rounding.

import os
import sys

import numpy as np

for _p in ("/opt/trn_rl_repo", os.path.expanduser("~/.axon_site/_ro/trn_rl_repo")):
    if os.path.isdir(_p) and _p not in sys.path:
        sys.path.insert(0, _p)

import concourse.bass as bass  # noqa: E402
import concourse.mybir as mybir  # noqa: E402
import concourse.tile as tile  # noqa: E402
from concourse import bacc  # noqa: E402
from concourse.bass_utils import run_bass_kernel_spmd  # noqa: E402
from concourse.masks import make_identity  # noqa: E402

F32 = mybir.dt.float32
F32R = mybir.dt.float32r
BF16 = mybir.dt.bfloat16
AF = mybir.ActivationFunctionType
ALU = mybir.AluOpType

B, S, H = 16, 4096, 1024
NCORES = 8
BPC = B // NCORES  # batches per core
SC = 512  # s-chunk (one PSUM bank at fp32)
NSC = S // SC  # 8
NHC = H // 128  # 8 contraction chunks
NGC = H // 128  # 8 output (g) chunks

_CACHE = {}
LAST_RESULTS = None


def _build():
    nc = bacc.Bacc(
        "TRN2", target_bir_lowering=False, debug=False, num_devices=NCORES
    )
    kT = nc.dram_tensor("kt_in", [BPC, H, S], BF16, kind="ExternalInput").ap()
    vT = nc.dram_tensor("vt_in", [BPC, H, S], F32, kind="ExternalInput").ap()
    wkT = nc.dram_tensor("wkt_in", [H, H], BF16, kind="ExternalInput").ap()
    vw = nc.dram_tensor("vw_in", [H], F32, kind="ExternalInput").ap()
    qkb = nc.dram_tensor("qkb_in", [BPC, H], F32, kind="ExternalInput").ap()
    madd = nc.dram_tensor("madd_in", [BPC, S], F32, kind="ExternalInput").ap()
    ctx_o = nc.dram_tensor("ctx_out", [BPC, H], F32, kind="ExternalOutput").ap()
    attn_o = nc.dram_tensor("attn_out", [BPC, S], F32, kind="ExternalOutput").ap()

    with tile.TileContext(nc) as tc:
        with (
            tc.tile_pool(name="consts", bufs=1) as consts,
            tc.tile_pool(name="kpool", bufs=2) as kpool,
            tc.tile_pool(name="vpool", bufs=6) as vpool,
            tc.tile_pool(name="thpool", bufs=3) as thpool,
            tc.tile_pool(name="rows", bufs=2) as rows,
            tc.tile_pool(name="bcpool", bufs=8) as bcpool,
            tc.tile_pool(name="misc", bufs=2) as misc,
            tc.tile_pool(name="pz", bufs=2, space="PSUM") as pz,
            tc.tile_pool(name="ps", bufs=2, space="PSUM") as ps,
            tc.tile_pool(name="pb", bufs=2, space="PSUM") as pb,
            tc.tile_pool(name="pt", bufs=1, space="PSUM") as pt,
        ):
            # ---- constants ----
            wk_sb = consts.tile([128, NHC, H], BF16, name="wk_sb")
            nc.sync.dma_start(
                out=wk_sb, in_=wkT.rearrange("(hc p) g -> p hc g", p=128)
            )
            vw_sb = consts.tile([128, NGC], F32R, name="vw_sb")
            nc.sync.dma_start(
                out=vw_sb, in_=vw.rearrange("(gc p) -> p gc", p=128).bitcast(F32R)
            )
            qkb_sb = consts.tile([128, BPC, NGC], F32, name="qkb_sb")
            nc.sync.dma_start(
                out=qkb_sb, in_=qkb.rearrange("b (gc p) -> p b gc", p=128)
            )
            ident = consts.tile([128, 128], F32, name="ident")
            make_identity(nc, ident)
            ones_f = consts.tile([1, 128], F32, name="ones_f")
            nc.vector.memset(ones_f, 1.0)
            ones_r = consts.tile([1, 128], F32R, name="ones_r")
            nc.vector.tensor_copy(ones_r, ones_f)
            ctx_cols = consts.tile([128, BPC, NHC], F32, name="ctx_cols")

            # Deferred PE work: score / broadcast matmuls depend on ScalarE /
            # DVE results; emitting them a little late keeps the in-order PE
            # queue from stalling on cross-engine latency.
            pending = []

            def pop_pending():
                if pending:
                    pending.pop(0)()

            for b in range(BPC):
                scores = rows.tile([1, S], F32, tag="scores", name=f"scores_{b}")
                madd_sb = rows.tile([1, S], F32, tag="madd", name=f"madd_{b}")
                nc.sync.dma_start(out=madd_sb, in_=madd[b][None])
                parts_b = [
                    misc.tile(
                        [128, NSC], F32, tag="parts", bufs=2 * NHC,
                        name=f"parts_{b}_{hc}",
                    )
                    for hc in range(NHC)
                ]

                for sc in range(NSC):
                    kt = kpool.tile(
                        [128, NHC, SC], BF16, tag="kt", bufs=3, name=f"kt_{b}_{sc}"
                    )
                    nc.sync.dma_start(
                        out=kt,
                        in_=kT[b, :, sc * SC:(sc + 1) * SC].rearrange(
                            "(hc p) s -> p hc s", p=128
                        ),
                    )
                    sc_ps = ps.tile([1, SC], F32, tag="scps", name=f"scps_{b}_{sc}")
                    for gc in range(NGC):
                        z_ps = pz.tile(
                            [128, SC], F32, tag="zps", name=f"z_{b}_{sc}_{gc}"
                        )
                        for hc in range(NHC):
                            nc.tensor.matmul(
                                z_ps,
                                lhsT=wk_sb[:, hc, gc * 128:(gc + 1) * 128],
                                rhs=kt[:, hc, :],
                                start=(hc == 0),
                                stop=(hc == NHC - 1),
                            )
                        th = thpool.tile(
                            [128, SC], F32R, tag="th", name=f"th_{b}_{sc}_{gc}"
                        )
                        nc.scalar.activation(
                            th, z_ps, AF.Tanh, bias=qkb_sb[:, b, gc:gc + 1], scale=1.0
                        )

                        def score_mm(b=b, sc=sc, gc=gc, th=th, sc_ps=sc_ps,
                                     scores=scores, madd_sb=madd_sb):
                            nc.tensor.matmul(
                                sc_ps,
                                lhsT=vw_sb[:, gc:gc + 1],
                                rhs=th,
                                start=(gc == 0),
                                stop=(gc == NGC - 1),
                                skip_group_check=True,
                            )
                            if gc == NGC - 1:
                                nc.vector.tensor_add(
                                    scores[:, sc * SC:(sc + 1) * SC],
                                    sc_ps,
                                    madd_sb[:, sc * SC:(sc + 1) * SC],
                                )

                        pop_pending()
                        pending.append(score_mm)

                attn_r = rows.tile([1, S], F32R, tag="attnr", name=f"attnr_{b}")

                def softmax_item(b=b, scores=scores, attn_r=attn_r):
                    denom = rows.tile([1, 1], F32, tag="denom", name=f"den_{b}")
                    nc.scalar.activation(scores, scores, AF.Exp, accum_out=denom)
                    recip = rows.tile([1, 1], F32, tag="recip", name=f"rec_{b}")
                    nc.vector.reciprocal(recip, denom)
                    nc.vector.tensor_scalar_mul(attn_r, scores, recip)
                    nc.sync.dma_start(out=attn_o[b][None], in_=attn_r.bitcast(F32))

                pending.append(softmax_item)

                for sc in range(NSC):
                    def ctx_item(b=b, sc=sc, attn_r=attn_r, parts_b=parts_b):
                        bc_ps = pb.tile(
                            [128, SC], F32, tag="bcps", name=f"bcps_{b}_{sc}"
                        )
                        nc.tensor.matmul(
                            bc_ps,
                            lhsT=ones_r,
                            rhs=attn_r[:, sc * SC:(sc + 1) * SC],
                            start=True,
                            stop=True,
                        )
                        bc_sb = bcpool.tile(
                            [128, SC], F32, tag="bc", name=f"bc_{b}_{sc}"
                        )
                        nc.scalar.copy(bc_sb, bc_ps)
                        for hc in range(NHC):
                            vt = vpool.tile(
                                [128, SC], F32, tag="vt", name=f"vt_{b}_{sc}_{hc}"
                            )
                            nc.sync.dma_start(
                                out=vt,
                                in_=vT[b, hc * 128:(hc + 1) * 128,
                                       sc * SC:(sc + 1) * SC],
                            )
                            junk = vpool.tile(
                                [128, SC], F32, tag="junk", bufs=2,
                                name=f"junk_{b}_{sc}_{hc}",
                            )
                            # one fused DVE pass: junk = vt*bc, accum_out
                            # gives the s-partial of ctx
                            nc.vector.scalar_tensor_tensor(
                                out=junk, in0=vt, scalar=1.0, in1=bc_sb,
                                op0=ALU.mult, op1=ALU.mult,
                                accum_out=parts_b[hc][:, sc:sc + 1],
                            )
                        if sc == NSC - 1:
                            for hc in range(NHC):
                                nc.vector.reduce_sum(
                                    ctx_cols[:, b, hc:hc + 1],
                                    parts_b[hc],
                                    axis=mybir.AxisListType.X,
                                )

                    pending.append(ctx_item)

            while pending:
                pending.pop(0)()

            # ctx_cols [128p, (b,hc)] -> [16, 128] so the DRAM store is
            # 16 contiguous 512B rows instead of a 4B-scatter.
            ct_ps = pt.tile([16, 128], F32, name="ct_ps")
            nc.tensor.transpose(
                ct_ps, ctx_cols.rearrange("p b h -> p (b h)"), ident
            )
            ctx_row = consts.tile([16, 128], F32, name="ctx_row")
            nc.vector.tensor_copy(ctx_row, ct_ps)
            nc.sync.dma_start(
                out=ctx_o.rearrange("b (hc j) -> (b hc) j", j=128), in_=ctx_row
            )

    nc.compile()
    return nc


def get_nc():
    if "nc" not in _CACHE:
        _CACHE["nc"] = _build()
    return _CACHE["nc"]


def make_in_maps(q, k, v, mask, Wq_w, Wq_b, Wk_w, Wk_b, v_w):
    q = np.asarray(q, np.float32)
    k = np.asarray(k, np.float32)
    v = np.asarray(v, np.float32)
    mask = np.asarray(mask)
    Wq_w = np.asarray(Wq_w, np.float32)
    Wq_b = np.asarray(Wq_b, np.float32)
    Wk_w = np.asarray(Wk_w, np.float32)
    Wk_b = np.asarray(Wk_b, np.float32)
    v_w = np.asarray(v_w, np.float32)

    import ml_dtypes

    # whole q path on host: [B,H], folded with both biases
    qkb = q[:, 0, :] @ Wq_w.T + Wq_b + Wk_b
    kT = np.ascontiguousarray(k.transpose(0, 2, 1)).astype(ml_dtypes.bfloat16)
    vT = np.ascontiguousarray(v.transpose(0, 2, 1))
    wkT = np.ascontiguousarray(Wk_w.T).astype(ml_dtypes.bfloat16)
    madd = np.where(mask, np.float32(0.0), np.float32(-1e9)).astype(np.float32)
    vww = np.ascontiguousarray(v_w[0])

    in_maps = []
    for c in range(NCORES):
        sl = slice(c * BPC, (c + 1) * BPC)
        in_maps.append(
            {
                "kt_in": np.ascontiguousarray(kT[sl]),
                "vt_in": np.ascontiguousarray(vT[sl]),
                "wkt_in": wkT,
                "vw_in": vww,
                "qkb_in": np.ascontiguousarray(qkb[sl]),
                "madd_in": np.ascontiguousarray(madd[sl]),
            }
        )
    return in_maps


def kernel(q, k, v, mask, Wq_w, Wq_b, Wk_w, Wk_b, v_w):
    global LAST_RESULTS
    in_maps = make_in_maps(q, k, v, mask, Wq_w, Wq_b, Wk_w, Wk_b, v_w)
    nc = get_nc()
    res = run_bass_kernel_spmd(nc, in_maps, core_ids=list(range(NCORES)))
    LAST_RESULTS = res
    ctx = np.concatenate(
        [res.results[c]["ctx_out"] for c in range(NCORES)], axis=0
    ).astype(np.float32)
    attn = np.concatenate(
        [res.results[c]["attn_out"] for c in range(NCORES)], axis=0
    ).astype(np.float32)
    return ctx, attn
